# revision 30
# baseline (speedup 1.0000x reference)
"""Causal self-attention (B=4, T=2048, D=1024, H=16, hd=64) on 8 TRN2 NeuronCores.

Sharding: core c handles batch b = c % 4 and head-half = c // 4 (8 heads each).
Each core computes, for its (batch, 8 heads):
    qkv projection -> causal attention -> partial output projection (yT).
Host gathers: y[b] = (yT[core b] + yT[core b+4]).T + b_proj.

Device design (per core), v2 (bf16 + dense-PE schedule):
  - all matmul operands bf16 (fp32 PSUM accumulate); weights/x DMA'd as bf16
    so initial HBM traffic halves and LDWEIGHTS gets FWL
  - unified 512-wide chunks: phase-1 chunk j feeds attention chunk j 1:1
  - attention computes S^T [k, q] tiles; two heads row-packed on the PE at
    tile_position (0,0)/(64,0) (contraction = hd = 64, concurrent)
  - softmax without max-subtraction (logits are small); exp on ScalarE with
    the 1/sqrt(hd) scale fused; causal masking via gpsimd affine_select on
    diagonal tiles only
  - V' = [V | 1] trick: wv gets a zero 65th column per head and bv' a 1.0, so
    each PV matmul (M=65) also produces the softmax denominator in row 64
  - normalize: DVE reciprocal, PE partition-broadcast (sel matrix), DVE mult
  - PE-dense emission: the attention kt-loop is software-pipelined (QKT of
    step kt+1 is emitted before PV of step kt, so the PE never head-of-line
    blocks on ACT's exp), and phase-1 / output-projection matmuls of
    neighboring chunks are injected as filler units between attention steps.
    This keeps TensorE busy end-to-end so the HAM clock gate stays at 8/8.
"""

import os
import sys
from contextlib import ExitStack

import numpy as np

sys.path.insert(0, "/opt/trn_rl_repo")

import concourse.bass as bass  # noqa: E402
import concourse.tile as tile  # noqa: E402
from concourse import bacc, mybir  # noqa: E402

f32 = mybir.dt.float32
f32r = mybir.dt.float32r
bf16 = mybir.dt.bfloat16
EXP = mybir.ActivationFunctionType.Exp

B, T, D = 4, 2048, 1024
H, HD = 16, 64
HDP = HD + 1       # 65: head dim + ones column
HPC = 8            # heads per core
NP = 4             # head pairs per core
NCORES = 8
CH = 512           # unified chunk width (phase-1 t-chunk == attention q-chunk)
NCH = T // CH      # 4
NKT = T // 128     # 16 k-tiles
VW = HPC * HDP     # 520: V' width
VH = VW // 2       # 260: V' half width (one matmul's N)


def build_program():
    nc = bacc.Bacc("TRN2", target_bir_lowering=False, debug=False)

    xT = nc.dram_tensor("xT", [D, T], bf16, kind="ExternalInput").ap()
    wqk = nc.dram_tensor("wqk", [D, 2 * HPC * HD], bf16, kind="ExternalInput").ap()
    wv = nc.dram_tensor("wv", [D, VW], bf16, kind="ExternalInput").ap()
    wp = nc.dram_tensor("wp", [HPC * HD, D], bf16, kind="ExternalInput").ap()
    bqk = nc.dram_tensor("bqk", [2 * HPC * HD, 1], f32, kind="ExternalInput").ap()
    bv = nc.dram_tensor("bv", [128, VW], f32, kind="ExternalInput").ap()
    sel = nc.dram_tensor("sel", [33, 128], bf16, kind="ExternalInput").ap()
    yT = nc.dram_tensor("yT", [D, T], bf16, kind="ExternalOutput").ap()

    with tile.TileContext(nc) as tc:
        with ExitStack() as ctx:
            _build(ctx, tc, xT, wqk, wv, wp, bqk, bv, sel, yT)
    nc.compile()
    return nc


def _build(ctx, tc, xT, wqk, wv, wp, bqk, bv, sel, yT):
    nc = tc.nc

    persist = ctx.enter_context(tc.tile_pool(name="persist", bufs=1))
    wqk_pool = ctx.enter_context(tc.tile_pool(name="wqk_pool", bufs=4))
    w16 = ctx.enter_context(tc.tile_pool(name="w16", bufs=1))
    xc_pool = ctx.enter_context(tc.tile_pool(name="xc_pool", bufs=3))
    es_pool = ctx.enter_context(tc.tile_pool(name="es_pool", bufs=6))
    pin_pool = ctx.enter_context(tc.tile_pool(name="pin_pool", bufs=3))
    ysb_pool = ctx.enter_context(tc.tile_pool(name="ysb_pool", bufs=4))
    small = ctx.enter_context(tc.tile_pool(name="small", bufs=6))

    p1ps = ctx.enter_context(tc.tile_pool(name="p1ps", bufs=2, space="PSUM"))
    s_ps_pool = ctx.enter_context(tc.tile_pool(name="s_ps_pool", bufs=2, space="PSUM"))
    pv_ps_pool = ctx.enter_context(tc.tile_pool(name="pv_ps_pool", bufs=2, space="PSUM"))

    # ---- persistent tensors ----
    qT = persist.tile([128, NP, T], bf16, tag="qT")     # [2 heads x 64 dims, pair, t]
    kT = persist.tile([128, NP, T], bf16, tag="kT")
    V = persist.tile([128, NKT, VW], bf16, tag="V")     # [t in tile, k-tile, h*65+d]

    # ---- x chunk loads (split in two halves so first matmuls start early) ----
    xc_tiles = {}

    def emit_xc(j):
        xc = xc_pool.tile([128, 8, CH], bf16, name=f"xc{j}", tag="xc")
        xv = xT.rearrange("(d p) t -> p d t", p=128)[:, :, j * CH:(j + 1) * CH]
        nc.sync.dma_start(out=xc[:, 0:4, :], in_=xv[:, 0:4, :])
        nc.sync.dma_start(out=xc[:, 4:8, :], in_=xv[:, 4:8, :])
        xc_tiles[j] = xc

    # ---- PE warm-up: ~30 matmuls on a memset scratch tile, emitted ahead
    # of everything so the HAM clock gate reaches 8/8 while the input DMAs
    # are still in flight (results go to an unread PSUM scratch) ----
    warm = small.tile([128, CH], bf16, tag="warm", bufs=1)
    nc.gpsimd.memset(warm, 0.0)
    warm_ps = pv_ps_pool.tile([128, CH], f32, name="warm_ps", tag="pv")
    for i in range(30):
        nc.tensor.matmul(warm_ps, warm[:, 0:128], warm, start=True, stop=True)

    # ---- critical-path loads: xc0 + wqk (first matmuls gate on these),
    # split finely so they spread across many DMA queues ----
    emit_xc(0)
    wqk_sb = []
    for s in range(4):
        w = wqk_pool.tile([128, 2, 2 * HPC * HD], bf16, name=f"wqk_sb{s}", tag="wqk")
        wsrc = wqk[2 * s * 128:(2 * s + 2) * 128, :].rearrange("(i p) m -> p i m", p=128)
        nc.sync.dma_start(out=w[:, 0:1, :], in_=wsrc[:, 0:1, :])
        nc.sync.dma_start(out=w[:, 1:2, :], in_=wsrc[:, 1:2, :])
        wqk_sb.append(w)
    bqk_sb = small.tile([128, 8], f32, tag="bqk_sb", bufs=1)
    nc.sync.dma_start(out=bqk_sb, in_=bqk.rearrange("(m p) o -> p (m o)", p=128))

    # remaining loads are emitted later (behind the critical ones)
    wv_sb = w16.tile([128, 8, VW], bf16, tag="wv_sb")
    bv_sb = small.tile([128, VW], f32, tag="bv_sb", bufs=1)
    sel_sb = small.tile([128, 128], bf16, tag="sel_sb", bufs=1)
    zreg = nc.gpsimd.to_reg(0.0)

    wp_sb = None  # loaded lazily after phase-1 chunk 0 is emitted

    # ================= phase 1 unit generators (chunk jc) =================
    # Each unit is ~0.9us of PE work: half a qk m-tile (4 MMs) or one V'
    # (t-tile, group) accumulation (8 MMs N=260).
    def p1_units(jc, mts=range(8), v_part=True):
        xc = xc_tiles[jc]
        units = []
        state = {}

        def qk_first(mt):
            def emit():
                ps = p1ps.tile([128, CH], f32, name=f"qk_{jc}_{mt}", tag="p1")
                state[mt] = ps
                for dt in range(4):
                    nc.tensor.matmul(
                        ps, wqk_sb[dt // 2][:, dt % 2, mt * 128:(mt + 1) * 128],
                        xc[:, dt, :], start=(dt == 0), stop=False)
            return emit

        def qk_second(mt):
            def emit():
                ps = state.pop(mt)
                for dt in range(4, 8):
                    nc.tensor.matmul(
                        ps, wqk_sb[dt // 2][:, dt % 2, mt * 128:(mt + 1) * 128],
                        xc[:, dt, :], start=False, stop=(dt == 7))
                dest = qT if mt < 4 else kT
                nc.vector.tensor_scalar_add(
                    dest[:, mt % 4, jc * CH:(jc + 1) * CH], ps, bqk_sb[:, mt:mt + 1])
            return emit

        def v_unit(tt, g):
            def emit():
                ps = p1ps.tile([128, VH], f32, name=f"v_{jc}_{tt}_{g}", tag="p1")
                for dt in range(8):
                    nc.tensor.matmul(
                        ps, xc[:, dt, tt * 128:(tt + 1) * 128],
                        wv_sb[:, dt, g * VH:(g + 1) * VH],
                        start=(dt == 0), stop=(dt == 7))
                t_idx = jc * 4 + tt
                nc.vector.tensor_tensor(
                    out=V[:, t_idx, g * VH:(g + 1) * VH], in0=ps,
                    in1=bv_sb[:, g * VH:(g + 1) * VH], op=mybir.AluOpType.add)
            return emit

        for mt in mts:
            units.append(qk_first(mt))
            units.append(qk_second(mt))
        if v_part:
            for g in range(2):
                for tt in range(4):
                    units.append(v_unit(tt, g))
        return units

    # ================= output projection units (chunk j) =================
    def proj_units(j, pin, s_tail_from=None):
        units = []

        def unit(mt):
            def emit():
                # units used as chunk-tail fillers draw PSUM from the s pool
                # (idle once the last QKT is done) so the scheduler never
                # defers them behind the finish for lack of a p1ps slot
                if s_tail_from is not None and mt >= s_tail_from:
                    y_ps = s_ps_pool.tile([128, CH], f32, name=f"y_{j}_{mt}", tag="s")
                else:
                    y_ps = p1ps.tile([128, CH], f32, name=f"y_{j}_{mt}", tag="p1")
                for p in range(NP):
                    nc.tensor.matmul(
                        y_ps, wp_sb[:, p, mt * 128:(mt + 1) * 128], pin[:, p, :],
                        start=(p == 0), stop=(p == NP - 1))
                y_sb = ysb_pool.tile([128, CH], bf16, name=f"ysb_{j}_{mt}", tag="ysb")
                nc.vector.tensor_copy(out=y_sb, in_=y_ps)
                nc.sync.dma_start(
                    out=yT[mt * 128:(mt + 1) * 128, j * CH:(j + 1) * CH], in_=y_sb)
            return emit

        for mt in range(8):
            units.append(unit(mt))
        return units

    # ================= attention chunk (software-pipelined) =================
    def make_gather(j):
        # sums rows live at partitions {0,32,64,96} (legal DVE bases); rest
        # memset to 1.0 so the batched in-place reciprocal stays finite
        gather = []
        for i in range(2):
            ga = small.tile([128, CH], f32, name=f"gather{j}_{i}", tag="ga", bufs=3)
            nc.gpsimd.memset(ga, 1.0)
            gather.append(ga)
        return gather

    def attn_chunk(j, pin, fillers, front=()):
        nkt = 4 * (j + 1)
        last = nkt - 1
        q0 = j * CH
        gather = make_gather(j)
        nsteps = NP * nkt
        # hold back fillers to pad the PE stream during the finish's
        # reciprocal wait (more for the last chunk: its finish gates the
        # final projection and a cold HAM there doubles proj time)
        tn = 5 if j == NCH - 1 else 4
        tail_fillers = fillers[-tn:] if len(fillers) > tn else []
        fillers = fillers[:-tn] if len(fillers) > tn else fillers
        front = list(front)
        # bias ~half a filler to each pair start: the new pair's PV(0) waits
        # on the old pair's PSUM drain
        bias = 0.5 if len(fillers) >= nsteps / 8 else 0.0
        rate = max(0.0, (len(fillers) - bias * NP) / nsteps)
        frac, fi = 0.0, 0
        step = 0
        pair_ps = {}
        pending = None

        def make_pv(p, kt, e, c0):
            hA, hB = 2 * p, 2 * p + 1

            def emit():
                if kt == 0:
                    pair_ps[p] = (
                        pv_ps_pool.tile([128, CH], f32, name=f"pvA_{j}_{p}", tag="pv"),
                        pv_ps_pool.tile([128, CH], f32, name=f"pvB_{j}_{p}", tag="pv"),
                    )
                pvA, pvB = pair_ps[p]
                eA = e[:, c0:CH]
                eB = e[:, CH + c0:2 * CH]
                nc.tensor.matmul(pvA[0:HDP, c0:CH], V[:, kt, hA * HDP:(hA + 1) * HDP],
                                 eA, start=(kt == 0), stop=(kt == last))
                nc.tensor.matmul(pvB[0:HDP, c0:CH], V[:, kt, hB * HDP:(hB + 1) * HDP],
                                 eB, start=(kt == 0), stop=(kt == last))
                if kt == last:
                    # drain PSUM: U rows (unnormalized out) on DVE; sums rows
                    # on the (idle-at-this-point) ACT engine so the finish's
                    # reciprocal isn't queued behind four serial DVE copies.
                    # For the chunk's final pair the denominator rows are the
                    # critical path into finish_half(1): split them across
                    # ACT+DVE and emit them before the big U drains.
                    ga = gather[p // 2]
                    r0 = 64 * (p % 2)
                    if p == NP - 1:
                        nc.scalar.copy(out=ga[r0:r0 + 1, :], in_=pvA[64:65, :])
                        nc.vector.tensor_copy(out=ga[r0 + 32:r0 + 33, :],
                                              in_=pvB[64:65, :])
                        nc.scalar.copy(out=pin[0:64, p, :], in_=pvA[0:64, :])
                        nc.scalar.copy(out=pin[64:128, p, :], in_=pvB[0:64, :])
                    else:
                        # mid-chunk pairs: everything on DVE — ACT must stay
                        # exp-only or the next pair's PV chain stalls
                        nc.vector.tensor_copy(out=pin[0:64, p, :], in_=pvA[0:64, :])
                        nc.vector.tensor_copy(out=pin[64:128, p, :], in_=pvB[0:64, :])
                        nc.vector.tensor_copy(out=ga[r0:r0 + 1, :], in_=pvA[64:65, :])
                        nc.vector.tensor_copy(out=ga[r0 + 32:r0 + 33, :],
                                              in_=pvB[64:65, :])
                    del pair_ps[p]
            return emit

        for p in range(NP):
            for kt in range(nkt):
                # o > 0 on diagonal tiles: columns [0:128*o) are fully masked,
                # so QKT/exp/PV all skip them (exact).
                o = max(0, kt - 4 * j)
                c0 = 128 * o
                W = CH - c0
                s = s_ps_pool.tile([128, 2 * CH], f32, name=f"s_{j}_{p}_{kt}", tag="s")
                nc.tensor.matmul(s[:, c0:CH], kT[0:64, p, kt * 128:(kt + 1) * 128],
                                 qT[0:64, p, q0 + c0:q0 + CH],
                                 start=True, stop=True, tile_position=(0, 0))
                nc.tensor.matmul(s[:, CH + c0:2 * CH], kT[64:128, p, kt * 128:(kt + 1) * 128],
                                 qT[64:128, p, q0 + c0:q0 + CH],
                                 start=True, stop=True, tile_position=(64, 0))
                e = es_pool.tile([128, 2 * CH], bf16, name=f"e_{j}_{p}_{kt}", tag="e")
                if o == 0:
                    nc.scalar.activation(e, s, EXP, scale=0.125)
                else:
                    # one strided activation covering both heads' valid ranges
                    sv = s.rearrange("p (h q) -> p h q", h=2)[:, :, c0:CH]
                    ev = e.rearrange("p (h q) -> p h q", h=2)[:, :, c0:CH]
                    nc.scalar.activation(ev, sv, EXP, scale=0.125)
                if kt >= 4 * j:
                    # staircase mask within the remaining width: keep col >= kl
                    ev2 = e.rearrange("p (h q) -> p h q", h=2)[:, :, c0:CH]
                    nc.gpsimd.affine_select(
                        ev2, ev2, pattern=[[0, 2], [1, W]],
                        compare_op=mybir.AluOpType.is_ge, fill=zreg,
                        base=0, channel_multiplier=-1)
                if pending is not None:
                    pending()
                pending = make_pv(p, kt, e, c0)
                if kt == 1:
                    frac += bias
                if p == 2 and kt == 3:
                    # pairs 0/1 fully drained two steps ago: normalize them
                    # now, overlapped with pair 2's compute; pad the PE stream
                    # with two fillers while DVE runs the reciprocal
                    def pad0():
                        nonlocal fi
                        n = 0
                        while n < 2 and fi < len(fillers):
                            fillers[fi]()
                            fi += 1
                            n += 1
                    finish_half(j, pin, gather, 0, pad=pad0)
                if step < len(front):
                    front[step]()
                frac += rate
                while frac >= 1.0 and fi < len(fillers):
                    fillers[fi]()
                    fi += 1
                    frac -= 1.0
                step += 1
        pending()
        while step < len(front):
            front[step]()
            step += 1
        while fi < len(fillers):
            fillers[fi]()
            fi += 1
        for f in tail_fillers:
            f()
        finish_half(j, pin, gather, 1)

    def finish_half(j, pin, gather, half, pad=None):
        # batched approx reciprocal (~18 correct bits): one in-place
        # [128, 512] op covers 2 heads' denominators
        ga = gather[half]
        nc.vector.reciprocal_approx_fast(out=ga, in_=ga)
        ga_bf = small.tile([128, CH], bf16, name=f"gab{j}_{half}", tag="gab", bufs=2)
        with nc.allow_low_precision(reason="bf16 reciprocal broadcast: benign"):
            nc.vector.tensor_copy(out=ga_bf, in_=ga)
        if pad is not None:
            pad()
        for p in (2 * half, 2 * half + 1):
            r0 = 64 * (p % 2)
            # partition-broadcast via PE: sel33 puts ga row r0 on partitions
            # 0..63 and row r0+32 on 64..127 (zero rows cancel the garbage)
            bc_ps = p1ps.tile([128, CH], f32, name=f"bcps_{j}_{p}", tag="p1")
            nc.tensor.matmul(bc_ps, sel_sb[r0:r0 + 33, :], ga_bf[r0:r0 + 33, :],
                             start=True, stop=True)
            nc.vector.tensor_tensor(out=pin[:, p, :], in0=pin[:, p, :],
                                    in1=bc_ps, op=mybir.AluOpType.mult)

    # ================= emission =================
    # chunk-0 qk first (gates only on xc0 + wqk), then the remaining loads,
    # then chunk-0 V' — so the PE starts as soon as the critical 3MB lands
    p1_0_qk = p1_units(0, v_part=False)
    for u in p1_0_qk[:4]:
        u()
    nc.sync.dma_start(out=wv_sb, in_=wv.rearrange("(d p) n -> p d n", p=128))
    nc.sync.dma_start(out=bv_sb, in_=bv)
    nc.sync.dma_start(out=sel_sb[0:33, :], in_=sel)
    nc.sync.dma_start(out=sel_sb[64:97, :], in_=sel)
    for u in p1_0_qk[4:]:
        u()
    emit_xc(1)
    for u in p1_units(0, mts=(), v_part=True):
        u()
    wp_sb = w16.tile([128, NP, D], bf16, tag="wp_sb")
    nc.sync.dma_start(out=wp_sb, in_=wp.rearrange("(k p) m -> p k m", p=128))

    def pin_tile(j):
        return pin_pool.tile([128, NP, CH], bf16, name=f"pin{j}", tag="pin")

    pin0 = pin_tile(0)
    emit_xc(2)
    attn_chunk(0, pin0, p1_units(1))
    pin1 = pin_tile(1)
    emit_xc(3)
    attn_chunk(1, pin1, p1_units(2))
    pin2 = pin_tile(2)
    attn_chunk(2, pin2, proj_units(0, pin0) + proj_units(1, pin1) + p1_units(3, mts=range(0, 4), v_part=False))
    pin3 = pin_tile(3)
    # attn(3)'s kts 12-15 need p1(3)'s kT m-tiles (mt 4-7) and V tiles 12-15;
    # front-load them at one unit per step so they land well before step 12.
    # proj(1)+proj(2) as spread fillers make attn(3) exactly PE-paced.
    front3 = p1_units(3, mts=range(4, 8), v_part=True)
    attn_chunk(3, pin3, proj_units(2, pin2, s_tail_from=3), front=front3)
    for u in proj_units(3, pin3):
        u()


# ======================= host side =======================

_NC_CACHE = None
LAST_RESULT = None


def _get_program():
    global _NC_CACHE
    if _NC_CACHE is None:
        _NC_CACHE = build_program()
    return _NC_CACHE


def shard_inputs(x, w_qkv, b_qkv, w_proj):
    import ml_dtypes
    bf = ml_dtypes.bfloat16
    x = np.asarray(x, dtype=np.float32)
    w_qkv = np.asarray(w_qkv, dtype=np.float32)
    b_qkv = np.asarray(b_qkv, dtype=np.float32)
    w_proj = np.asarray(w_proj, dtype=np.float32)
    sel_const = np.zeros((33, 128), dtype=np.float32)
    sel_const[0, 0:64] = 1.0
    sel_const[32, 64:128] = 1.0
    in_maps = []
    for c in range(NCORES):
        b = c % B
        half = c // B
        hs = half * (HPC * HD)  # 512
        wq = w_qkv[:, 0 * D + hs:0 * D + hs + HPC * HD]
        wk = w_qkv[:, 1 * D + hs:1 * D + hs + HPC * HD]
        wv_ = w_qkv[:, 2 * D + hs:2 * D + hs + HPC * HD]
        bq = b_qkv[0 * D + hs:0 * D + hs + HPC * HD]
        bk = b_qkv[1 * D + hs:1 * D + hs + HPC * HD]
        bv_ = b_qkv[2 * D + hs:2 * D + hs + HPC * HD]
        # V' = [V | 1]: wv gets a zero 65th column per head; bv' a 1.0 there
        wvp = np.zeros((D, VW), dtype=np.float32)
        bvp = np.zeros((VW,), dtype=np.float32)
        for h in range(HPC):
            wvp[:, h * HDP:h * HDP + HD] = wv_[:, h * HD:(h + 1) * HD]
            bvp[h * HDP:h * HDP + HD] = bv_[h * HD:(h + 1) * HD]
            bvp[h * HDP + HD] = 1.0
        in_maps.append({
            "xT": np.ascontiguousarray(x[b].T).astype(bf),
            "wqk": np.ascontiguousarray(np.concatenate([wq, wk], axis=1)).astype(bf),
            "wv": wvp.astype(bf),
            "wp": np.ascontiguousarray(w_proj[hs:hs + HPC * HD, :]).astype(bf),
            "bqk": np.ascontiguousarray(np.concatenate([bq, bk])[:, None]),
            "bv": np.ascontiguousarray(np.broadcast_to(bvp[None, :], (128, VW))),
            "sel": sel_const.astype(bf),
        })
    return in_maps


def kernel(x, w_qkv, b_qkv, w_proj, b_proj):
    global LAST_RESULT
    from concourse.bass_utils import run_bass_kernel_spmd

    nc = _get_program()
    in_maps = shard_inputs(x, w_qkv, b_qkv, w_proj)
    res = run_bass_kernel_spmd(nc, in_maps, list(range(NCORES)))
    LAST_RESULT = res
    b_proj = np.asarray(b_proj, dtype=np.float32)
    y = np.empty((B, T, D), dtype=np.float32)
    for b in range(B):
        yTfull = (res.results[b]["yT"].astype(np.float32)
                  + res.results[b + B]["yT"].astype(np.float32))
        y[b] = yTfull.T + b_proj[None, :]
    return y
